# revision 1
# baseline (speedup 1.0000x reference)
"""Trainium2 Bass kernel for the CKTGNN batch-of-DAGs encoder.

Strategy (pure data parallel, B=4096 split over 8 NeuronCores, 512 graphs/core):
  - one-hot inputs are built on device (iota + is_equal compares) and the
    per-vertex x-side GRU/gate contributions come from K=19 one-hot matmuls
    with biases folded into the one-hot columns,
  - the 12-step vertex scan keeps the GRU state batch-major; PE transposes
    produce the feature-major copies the matmuls need,
  - the masked predecessor sum (h_in) exploits that each vertex's gated
    message is written once: it is a running per-batch-scalar axpy over
    previously computed gated blocks (fused DVE scalar_tensor_tensor),
  - gi+gh pre-activations are fused in PSUM by accumulating the one-hot
    matmul and the W_hh matmul into the same banks,
  - the df scatter (last-write-wins) is a select chain expressed as
    df += onehot*(feats - df) with broadcast access patterns.

kernel(**inputs) takes the full unsharded inputs, shards batch over the 8
cores, runs the SPMD bass kernel, and concatenates the shard outputs.
"""

from contextlib import ExitStack

import numpy as np

import concourse.bass as bass
import concourse.tile as tile
from concourse import bacc, mybir
from concourse.bass_utils import run_bass_kernel_spmd
from concourse.masks import make_identity

NCORES = 8
B = 4096
BL = B // NCORES          # batch per core
CH = BL // 128            # 128-row chunks per core
MAX_N = 12
NT = 10
PP = 9
HID = 301
GI = 3 * HID              # 903
OUT_W = 112

f32 = mybir.dt.float32
bf16 = mybir.dt.bfloat16
i32 = mybir.dt.int32
OP = mybir.AluOpType
AF = mybir.ActivationFunctionType

# feature-dim k-chunks of the hidden state (offset, rows)
KCH = [(0, 128), (128, 128), (256, 45)]
# b_hh[0:602] is folded into wpreA; the hn-part bias comes from a K=1
# ones-column matmul accumulated into PSUM

_CACHE = {}


def _body(ctx, tc, nc, d, d_out):
    cs = lambda c: slice(128 * c, 128 * (c + 1))

    consts = ctx.enter_context(tc.tile_pool(name="consts", bufs=1))
    wp = ctx.enter_context(tc.tile_pool(name="wp", bufs=1))
    big = ctx.enter_context(tc.tile_pool(name="big", bufs=1))
    pin = ctx.enter_context(tc.tile_pool(name="pin", bufs=2))
    p_hin = ctx.enter_context(tc.tile_pool(name="p_hin", bufs=16))
    p_rz = ctx.enter_context(tc.tile_pool(name="p_rz", bufs=6))
    p_t = ctx.enter_context(tc.tile_pool(name="p_t", bufs=10))
    p_g = ctx.enter_context(tc.tile_pool(name="p_g", bufs=6))
    p_hv = ctx.enter_context(tc.tile_pool(name="p_hv", bufs=8))
    p_sg = ctx.enter_context(tc.tile_pool(name="p_sg", bufs=6))
    p_out = ctx.enter_context(tc.tile_pool(name="p_out", bufs=4))
    # PSUM: PR 2 x [128,1024] (2 banks each) + P1 3 x [128,512] (1 bank) = 7
    PR = ctx.enter_context(tc.tile_pool(name="PR", bufs=2, space="PSUM"))
    P1 = ctx.enter_context(tc.tile_pool(name="P1", bufs=4, space="PSUM"))

    mm = nc.tensor.matmul

    # ---------------- constants ----------------
    ident = consts.tile([128, 128], f32, name="ident", tag="ident")
    make_identity(nc, ident[:])
    ident_b = consts.tile([128, 128], bf16, name="ident_b", tag="ident_b")
    make_identity(nc, ident_b[:])
    cmpi = consts.tile([128, NT], i32, name="cmpi", tag="cmpi")
    nc.gpsimd.iota(cmpi[:], pattern=[[1, NT]], base=0, channel_multiplier=0)
    cmpf = consts.tile([128, NT], f32, name="cmpf", tag="cmpf")
    nc.vector.tensor_copy(cmpf[:], cmpi[:])
    zeros301 = consts.tile([128, HID], f32, name="zeros301", tag="zeros301")
    nc.vector.memset(zeros301[:], 0.0)

    # ---------------- weights ----------------
    wpreA = wp.tile([19, GI], bf16, name="wpreA", tag="wpreA")
    nc.sync.dma_start(wpreA[:], d["wpreA"][:])
    whh = []
    for k, (ko, kk) in enumerate(KCH):
        t = wp.tile([kk, GI], bf16, name=f"whh{k}", tag=f"whh{k}")
        nc.sync.dma_start(t[:], d["whh"][ko:ko + kk, :])
        whh.append(t)
    wpreI = wp.tile([19, HID], bf16, name="wpreI", tag="wpreI")
    nc.sync.dma_start(wpreI[:], d["wpreI"][:])
    wgm = []
    for k, (ko, kk) in enumerate(KCH[:2]):
        t = wp.tile([kk, 602], bf16, name=f"wgm{k}", tag=f"wgm{k}")
        nc.sync.dma_start(t[:], d["wgm"][ko:ko + kk, :])
        wgm.append(t)
    wBc = wp.tile([109, 602], bf16, name="wBc", tag="wBc")
    nc.sync.dma_start(wBc[:], d["wBcomb"][:])
    wfc = []
    for k, sz in enumerate([128, 128, 97]):
        t = wp.tile([sz, OUT_W], bf16, name=f"wfc{k}", tag=f"wfc{k}")
        nc.sync.dma_start(t[:], d["wfc"][128 * k:128 * k + sz, :])
        wfc.append(t)
    wdf1 = wp.tile([33, 16], bf16, name="wdf1", tag="wdf1")
    nc.sync.dma_start(wdf1[:], d["wdf1"][:])
    wdf2 = wp.tile([33, 8], bf16, name="wdf2", tag="wdf2")
    nc.sync.dma_start(wdf2[:], d["wdf2"][:])

    # ---------------- inputs, one-hot ----------------
    adj_f, feats, X19 = [], [], []
    for c in range(CH):
        ti = pin.tile([128, MAX_N], i32, name="ti", tag="ti")
        nc.sync.dma_start(ti[:], d["types"][cs(c), :])
        tf = pin.tile([128, MAX_N], f32, name="tf", tag="tf")
        nc.vector.tensor_copy(tf[:], ti[:])
        pi = pin.tile([128, MAX_N], i32, name="pi", tag="pi")
        nc.sync.dma_start(pi[:], d["paths"][cs(c), :])
        pf = pin.tile([128, MAX_N], f32, name="pf", tag="pf")
        nc.vector.tensor_copy(pf[:], pi[:])
        ai = pin.tile([128, MAX_N * MAX_N], i32, name="ai", tag="ai")
        nc.sync.dma_start(ai[:], d["adj"][cs(c), :])
        af = big.tile([128, MAX_N * MAX_N], f32, name=f"adj{c}", tag=f"adj{c}")
        nc.vector.tensor_copy(af[:], ai[:])
        adj_f.append(af)
        ft = big.tile([128, 3 * MAX_N], f32, name=f"feats{c}", tag=f"feats{c}")
        nc.sync.dma_start(ft[:], d["feats"][cs(c), :])
        feats.append(ft)

        x = big.tile([128, MAX_N * 19], f32, name=f"X19_{c}", tag=f"X19_{c}")
        x3 = x[:].rearrange("p (v j) -> p v j", j=19)
        nc.vector.tensor_tensor(
            x3[:, :, 0:NT],
            tf[:].unsqueeze(2).broadcast_to([128, MAX_N, NT]),
            cmpf[:, 0:NT].unsqueeze(1).broadcast_to([128, MAX_N, NT]),
            OP.is_equal,
        )
        nc.vector.tensor_tensor(
            x3[:, :, NT:19],
            pf[:].unsqueeze(2).broadcast_to([128, MAX_N, PP]),
            cmpf[:, 0:PP].unsqueeze(1).broadcast_to([128, MAX_N, PP]),
            OP.is_equal,
        )
        X19.append(x)

    # feature-major one-hots: XvT[v] = [19, BL] (types onehot rows 0:10,
    # paths onehot rows 10:19)
    XvT = []
    for v in range(MAX_N):
        pt = P1.tile([128, BL], f32, name="p1", tag="p1")
        for c in range(CH):
            nc.tensor.transpose(
                pt[0:19, cs(c)], X19[c][:, 19 * v:19 * v + 19], ident[:]
            )
        xt = big.tile([19, BL], bf16, name=f"XvT{v}", tag=f"XvT{v}")
        nc.any.tensor_copy(xt[:], pt[0:19, :])
        XvT.append(xt)

    # inn (x-side candidate pre-activation) for all vertices, bf16 in SBUF
    inn_all = []
    for c in range(CH):
        t = big.tile([128, MAX_N * HID], bf16, name=f"inn{c}", tag=f"inn{c}")
        inn_all.append(t)
    for v in range(MAX_N):
        for c in range(CH):
            pt = P1.tile([128, BL], f32, name="p1", tag="p1")
            mm(pt[:, 0:HID], XvT[v][:, cs(c)], wpreI[:],
               start=True, stop=True)
            nc.any.tensor_copy(inn_all[c][:, HID * v:HID * (v + 1)],
                               pt[:, 0:HID])

    # gated message store (written once per vertex, read by later vertices)
    G_all = [big.tile([128, 11 * HID], bf16, name=f"G{c}", tag=f"G{c}") for c in range(CH)]

    # feature-major h_in / h buffers (k-chunked)
    hiT = [big.tile([128, BL], bf16, name="hiT0", tag="hiT0"),
           big.tile([128, BL], bf16, name="hiT1", tag="hiT1"),
           big.tile([45, BL], bf16, name="hiT2", tag="hiT2")]
    for t in hiT:
        nc.vector.memset(t[:], 0.0)
    hvT = [big.tile([128, BL], bf16, name="hvT0", tag="hvT0"),
           big.tile([128, BL], bf16, name="hvT1", tag="hvT1")]
    # gates combo lhsT: rows 0:19 one-hot, rows 64:109 h_v tail (32-aligned),
    # rows 19:64 zeroed once (matching weight rows are zero)
    comboV = big.tile([109, BL], bf16, name="comboV", tag="comboV")
    nc.vector.memset(comboV[:], 0.0)
    # FC tail lhsT: rows 0:45 h-tail, 64:72 Hd, 96 ones (32-aligned writes)
    fcK2 = big.tile([97, BL], bf16, name="fcK2", tag="fcK2")
    nc.vector.memset(fcK2[:], 0.0)
    nc.vector.memset(fcK2[96:97, :], 1.0)

    # ---------------- the vertex scan ----------------
    for v in range(MAX_N):
        # x-side one-hot matmuls first: independent of the recurrence, they
        # keep the PE busy while the DVE grinds the h_in chains
        Rs = []
        for c in range(CH):
            R = PR.tile([128, 1024], f32, name="R", tag="R")
            xv = XvT[v][:, cs(c)]
            mm(R[:, 0:512], xv, wpreA[:, 0:512], start=True, stop=False)
            mm(R[:, 512:903], xv, wpreA[:, 512:903], start=True, stop=False)
            Rs.append(R)
        # masked sum of predecessor gated messages, then its transpose
        if v > 0:
            h_ins = []
            for c in range(CH):
                hi = p_hin.tile([128, HID], f32, name="hin", tag="hin")
                a = adj_f[c]
                nc.vector.tensor_scalar_mul(
                    hi[:], G_all[c][:, 0:HID],
                    a[:, MAX_N * v:MAX_N * v + 1])
                for u in range(1, v):
                    nc.vector.scalar_tensor_tensor(
                        hi[:], G_all[c][:, HID * u:HID * (u + 1)],
                        a[:, MAX_N * v + u:MAX_N * v + u + 1], hi[:],
                        op0=OP.mult, op1=OP.add)
                h_ins.append(hi)
            for k, (ko, kk) in enumerate(KCH):
                pt = P1.tile([128, BL], f32, name="p1", tag="p1")
                for c in range(CH):
                    nc.tensor.transpose(pt[0:kk, cs(c)],
                                        h_ins[c][:, ko:ko + kk], ident[:])
                nc.scalar.copy(hiT[k][0:kk, :], pt[0:kk, :])
        else:
            h_ins = [zeros301] * CH

        # gi+gh fused in PSUM, then the GRU cell elementwise
        hvs = []
        for c in range(CH):
            R = Rs[c]
            for k, (ko, kk) in enumerate(KCH):
                l = hiT[k][0:kk, cs(c)]
                last = k == 2
                mm(R[:, 0:512], l, whh[k][:, 0:512], start=False, stop=last)
                mm(R[:, 512:903], l, whh[k][:, 512:903],
                   start=False, stop=last)
            rz = p_rz.tile([128, 602], f32, name="rz", tag="rz")
            nc.scalar.activation(rz[:], R[:, 0:602], AF.Sigmoid)
            tn = p_t.tile([128, HID], f32, name="tn", tag="tn")
            nc.vector.tensor_mul(tn[:], rz[:, 0:HID], R[:, 602:903])
            tn2 = p_t.tile([128, HID], f32, name="tn2", tag="tn2")
            nc.vector.tensor_add(tn2[:], tn[:],
                                 inn_all[c][:, HID * v:HID * (v + 1)])
            g = p_g.tile([128, HID], f32, name="g", tag="g")
            nc.scalar.activation(g[:], tn2[:], AF.Tanh)
            t3 = p_t.tile([128, HID], f32, name="t3", tag="t3")
            nc.vector.tensor_sub(t3[:], h_ins[c][:], g[:])
            t4 = p_t.tile([128, HID], f32, name="t4", tag="t4")
            nc.vector.tensor_mul(t4[:], t3[:], rz[:, HID:602])
            hv = p_hv.tile([128, HID], f32, name="hv", tag="hv")
            nc.vector.tensor_add(hv[:], g[:], t4[:])
            hvs.append(hv)

        # transpose h_v to feature-major (step 11 k=2 lands in the FC tile)
        for k, (ko, kk) in enumerate(KCH):
            pt = P1.tile([128, BL], f32, name="p1", tag="p1")
            for c in range(CH):
                nc.tensor.transpose(pt[0:kk, cs(c)],
                                    hvs[c][:, ko:ko + kk], ident[:])
            if k == 2:
                if v == MAX_N - 1:
                    nc.scalar.copy(fcK2[0:45, :], pt[0:kk, :])
                else:
                    nc.scalar.copy(comboV[64:109, :], pt[0:kk, :])
            else:
                nc.scalar.copy(hvT[k][0:kk, :], pt[0:kk, :])

        # gate * mapper on [h_v, pos-onehot]; the one-hot rows (pos+bg via
        # wBcomb) and the h_v tail share one K=109 stationary tile
        if v < MAX_N - 1:
            nc.scalar.copy(comboV[0:19, :], XvT[v][:])
            for c in range(CH):
                Pg = P1.tile([128, BL], f32, name="p1", tag="p1")
                Pm = P1.tile([128, BL], f32, name="p1", tag="p1")
                cv = comboV[0:109, cs(c)]
                mm(Pg[:, 0:HID], cv, wBc[:, 0:HID], start=True, stop=False)
                mm(Pm[:, 0:HID], cv, wBc[:, HID:602], start=True, stop=False)
                for k, (ko, kk) in enumerate(KCH[:2]):
                    l = hvT[k][0:kk, cs(c)]
                    last = k == 1
                    mm(Pg[:, 0:HID], l, wgm[k][:, 0:HID],
                       start=False, stop=last)
                    mm(Pm[:, 0:HID], l, wgm[k][:, HID:602],
                       start=False, stop=last)
                sg = p_sg.tile([128, HID], f32, name="sg", tag="sg")
                nc.scalar.activation(sg[:], Pg[:, 0:HID], AF.Sigmoid)
                nc.vector.tensor_mul(G_all[c][:, HID * v:HID * (v + 1)],
                                     sg[:], Pm[:, 0:HID])

    # ---------------- df scatter (last write wins) + df MLP ----------------
    dfT = big.tile([33, BL], bf16, name="dfT", tag="dfT")
    nc.vector.memset(dfT[:], 0.0)
    nc.vector.memset(dfT[32:33, :], 1.0)
    for c in range(CH):
        df = big.tile([128, 27], f32, name=f"df{c}", tag=f"df{c}")
        nc.vector.memset(df[:], 0.0)
        df3 = df[:].rearrange("p (q j) -> p q j", j=3)
        for v in range(MAX_N):
            f3 = feats[c][:, 3 * v:3 * v + 3].unsqueeze(1) \
                .broadcast_to([128, PP, 3])
            oh = X19[c][:, 19 * v + NT:19 * v + 19].unsqueeze(2) \
                .broadcast_to([128, PP, 3])
            s = p_t.tile([128, 27], f32, name="dfs", tag="dfs")
            s3 = s[:].rearrange("p (q j) -> p q j", j=3)
            nc.gpsimd.tensor_sub(s3, f3, df3)
            w = p_t.tile([128, 27], f32, name="dfw", tag="dfw")
            w3 = w[:].rearrange("p (q j) -> p q j", j=3)
            nc.gpsimd.tensor_mul(w3, oh, s3)
            nc.gpsimd.tensor_add(df3, df3, w3)
        pt = P1.tile([128, BL], f32, name="p1", tag="p1")
        nc.tensor.transpose(pt[0:27, cs(c)], df[:], ident[:])
        if c == CH - 1:
            pass
        # copy chunk block into dfT
        nc.any.tensor_copy(dfT[0:27, cs(c)], pt[0:27, cs(c)])

    pd1 = P1.tile([128, BL], f32, name="p1", tag="p1")
    mm(pd1[0:16, :], wdf1[:], dfT[:], start=True, stop=True)
    r1T = big.tile([33, BL], bf16, name="r1T", tag="r1T")
    nc.vector.memset(r1T[:], 0.0)
    nc.vector.memset(r1T[32:33, :], 1.0)
    nc.scalar.activation(r1T[0:16, :], pd1[0:16, :], AF.Relu)
    pd2 = P1.tile([128, BL], f32, name="p1", tag="p1")
    mm(pd2[0:8, :], wdf2[:], r1T[:], start=True, stop=True)
    nc.any.tensor_copy(fcK2[64:72, :], pd2[0:8, :])

    # ---------------- final fully-connected (mu | logvar) ----------------
    for c in range(CH):
        po = P1.tile([128, BL], f32, name="p1", tag="p1")
        mm(po[:, 0:OUT_W], hvT[0][:, cs(c)], wfc[0][:], start=True, stop=False)
        mm(po[:, 0:OUT_W], hvT[1][:, cs(c)], wfc[1][:], start=False, stop=False)
        mm(po[:, 0:OUT_W], fcK2[:, cs(c)], wfc[2][:], start=False, stop=True)
        ob = p_out.tile([128, OUT_W], f32, name="ob", tag="ob")
        nc.any.tensor_copy(ob[:], po[:, 0:OUT_W])
        nc.sync.dma_start(d_out[cs(c), :], ob[:])


def build_nc():
    nc = bacc.Bacc("TRN2", target_bir_lowering=False, debug=False,
                   num_devices=NCORES)
    d = {}
    for name, shape, dt in [
        ("types", [BL, MAX_N], i32),
        ("paths", [BL, MAX_N], i32),
        ("adj", [BL, MAX_N * MAX_N], i32),
        ("feats", [BL, 3 * MAX_N], f32),
        ("wpreA", [19, GI], bf16),
        ("wpreB", [19, 602], bf16),
        ("whh", [HID, GI], bf16),
        ("wpreI", [19, HID], bf16),
        ("wgm", [HID, 602], bf16),
        ("wBcomb", [109, 602], bf16),
        ("wAcomb", [109, GI], bf16),
        ("wfc", [353, OUT_W], bf16),
        ("wdf1", [33, 16], bf16),
        ("wdf2", [33, 8], bf16),
    ]:
        d[name] = nc.dram_tensor(name, shape, dt, kind="ExternalInput").ap()
    d_out = nc.dram_tensor("out", [BL, OUT_W], f32, kind="ExternalOutput").ap()
    with tile.TileContext(nc) as tc:
        with ExitStack() as ctx:
            _body(ctx, tc, nc, d, d_out)
    nc.compile()
    return nc


def prepack(inputs):
    ii = {k: np.asarray(v) for k, v in inputs.items()}
    W_ih, b_ih = ii["W_ih"].astype(np.float32), ii["b_ih"].astype(np.float32)
    Wg, bg = ii["Wg"].astype(np.float32), ii["bg"].astype(np.float32)
    Wm = ii["Wm"].astype(np.float32)
    b_hh = ii["b_hh"].astype(np.float32)
    # scan-side one-hot weights: gi r/z parts (+b_ih+b_hh) in [0:602];
    # [602:903] carries only b_hh's candidate part (inn itself is precomputed
    # separately via wpreI)
    wpreA = W_ih.T.copy()
    wpreA[:, 602:903] = 0.0
    wpreA[:NT, 0:602] += (b_ih + b_hh)[None, 0:602]
    wpreA[:NT, 602:903] += b_hh[None, 602:903]
    wpreI = W_ih.T[:, 602:903].copy()
    wpreI[:NT] += b_ih[None, 602:903]
    wpreB = np.zeros((19, 602), np.float32)
    wpreB[NT:19, 0:HID] = Wg[:, HID:HID + PP].T + bg[None, :]
    wpreB[NT:19, HID:602] = Wm[:, HID:HID + PP].T
    whh = ii["W_hh"].astype(np.float32).T.copy()
    wgm = np.zeros((HID, 602), np.float32)
    wgm[:, 0:HID] = Wg[:, 0:HID].T
    wgm[:, HID:602] = Wm[:, 0:HID].T
    wBcomb = np.zeros((109, 602), np.float32)
    wBcomb[0:19] = wpreB
    wBcomb[64:109] = wgm[256:301]
    wAcomb = np.zeros((109, GI), np.float32)
    wAcomb[0:19] = wpreA
    wAcomb[64:109] = whh[256:301]
    # FC lhsT rows: [0:256) = h dims 0:256 (two 128-chunks); tail chunk of 97
    # rows: 0:45 h-tail, 64:72 Hd, 96 biases (matches fcK2 on-device layout)
    wfcT1 = ii["W_fc1"].astype(np.float32).T   # [309, 56]
    wfcT2 = ii["W_fc2"].astype(np.float32).T
    wfc = np.zeros((353, OUT_W), np.float32)
    wfc[0:256, 0:56] = wfcT1[0:256]
    wfc[0:256, 56:112] = wfcT2[0:256]
    tail = np.zeros((97, OUT_W), np.float32)
    tail[0:45, 0:56] = wfcT1[256:301]
    tail[0:45, 56:112] = wfcT2[256:301]
    tail[64:72, 0:56] = wfcT1[301:309]
    tail[64:72, 56:112] = wfcT2[301:309]
    tail[96, 0:56] = ii["b_fc1"].astype(np.float32)
    tail[96, 56:112] = ii["b_fc2"].astype(np.float32)
    wfc[256:353] = tail
    wdf1 = np.zeros((33, 16), np.float32)
    wdf1[0:27] = ii["W_df1"].astype(np.float32).T
    wdf1[32] = ii["b_df1"].astype(np.float32)
    wdf2 = np.zeros((33, 8), np.float32)
    wdf2[0:16] = ii["W_df2"].astype(np.float32).T
    wdf2[32] = ii["b_df2"].astype(np.float32)
    import ml_dtypes
    out = dict(wpreA=wpreA, wpreB=wpreB, wpreI=wpreI, whh=whh, wgm=wgm,
               wBcomb=wBcomb, wAcomb=wAcomb,
               wfc=wfc, wdf1=wdf1, wdf2=wdf2)
    return {k: v.astype(ml_dtypes.bfloat16) for k, v in out.items()}


def shard_inputs(inputs):
    ii = {k: np.asarray(v) for k, v in inputs.items()}
    w = prepack(ii)
    maps = []
    for i in range(NCORES):
        sl = slice(i * BL, (i + 1) * BL)
        m = dict(
            types=np.ascontiguousarray(ii["types"][sl]).astype(np.int32),
            paths=np.ascontiguousarray(ii["paths"][sl]).astype(np.int32),
            adj=np.ascontiguousarray(
                ii["adj_raw"][sl].reshape(BL, MAX_N * MAX_N)).astype(np.int32),
            feats=np.ascontiguousarray(
                ii["feats"][sl].reshape(BL, 3 * MAX_N)).astype(np.float32),
            **w,
        )
        maps.append(m)
    return maps


def get_nc():
    if "nc" not in _CACHE:
        _CACHE["nc"] = build_nc()
    return _CACHE["nc"]


def kernel(**inputs):
    nc = get_nc()
    maps = shard_inputs(inputs)
    res = run_bass_kernel_spmd(nc, maps, list(range(NCORES)))
    out = np.concatenate([res.results[i]["out"] for i in range(NCORES)], axis=0)
    return np.ascontiguousarray(out.astype(np.float32))



# revision 8
# speedup vs baseline: 1.2500x; 1.2500x over previous
"""Trainium2 Bass kernel for the CKTGNN batch-of-DAGs encoder.

Strategy (pure data parallel, B=4096 split over 8 NeuronCores, 512 graphs/core):
  - one-hot inputs are built on device (iota + is_equal compares, bf16); the
    per-vertex one-hot rows ride as extra K-rows inside the h-side matmul's
    third k-chunk, so the x-side contribution (incl. biases) is free,
  - the 12-step vertex scan keeps the GRU state batch-major in bf16; bf16 PE
    transposes (1 cyc/row) produce the feature-major copies the matmuls need,
  - the masked predecessor sum (h_in) is a running per-batch-scalar axpy over
    previously computed gated blocks (fused DVE scalar_tensor_tensor, bf16),
  - a shared 109-row combo tile holds [h_in tail | one-hot | h_v tail]; the
    gh matmul reads rows 0:64, the gate matmul rows 45:109,
  - the df scatter (last-write-wins) runs on GpSimd overlapped with the scan.

kernel(**inputs) takes the full unsharded inputs, shards batch over the 8
cores, runs the SPMD bass kernel, and concatenates the shard outputs.
"""

from contextlib import ExitStack

import numpy as np

import concourse.bass as bass
import concourse.tile as tile
from concourse import bacc, mybir
from concourse.bass_utils import run_bass_kernel_spmd
from concourse.masks import make_identity

NCORES = 8
B = 4096
BL = B // NCORES          # batch per core
CH = BL // 128            # 128-row chunks per core
MAX_N = 12
NT = 10
PP = 9
HID = 301
GI = 3 * HID              # 903
OUT_W = 112

f32 = mybir.dt.float32
bf16 = mybir.dt.bfloat16
i32 = mybir.dt.int32
OP = mybir.AluOpType
AF = mybir.ActivationFunctionType

# feature-dim k-chunks of the hidden state (offset, rows); the 45-row tail
# shares a 64-row combo tile with the 19 one-hot rows
KCH = [(0, 128), (128, 128), (256, 45)]

_CACHE = {}


def _body(ctx, tc, nc, d, d_out):
    cs = lambda c: slice(128 * c, 128 * (c + 1))

    consts = ctx.enter_context(tc.tile_pool(name="consts", bufs=1))
    wp = ctx.enter_context(tc.tile_pool(name="wp", bufs=1))
    big = ctx.enter_context(tc.tile_pool(name="big", bufs=1))
    pin = ctx.enter_context(tc.tile_pool(name="pin", bufs=2))
    p_hin = ctx.enter_context(tc.tile_pool(name="p_hin", bufs=16))
    p_rz = ctx.enter_context(tc.tile_pool(name="p_rz", bufs=6))
    p_t = ctx.enter_context(tc.tile_pool(name="p_t", bufs=10))
    p_g = ctx.enter_context(tc.tile_pool(name="p_g", bufs=6))
    p_hv = ctx.enter_context(tc.tile_pool(name="p_hv", bufs=8))
    p_sg = ctx.enter_context(tc.tile_pool(name="p_sg", bufs=6))
    p_inn = ctx.enter_context(tc.tile_pool(name="p_inn", bufs=8))
    p_out = ctx.enter_context(tc.tile_pool(name="p_out", bufs=4))
    p_cmb = ctx.enter_context(tc.tile_pool(name="p_cmb", bufs=2))
    # PSUM: PB 3 x [128,1024] f32 (2 banks each) + PT 2 x [128,512] bf16
    PB = ctx.enter_context(tc.tile_pool(name="PB", bufs=3, space="PSUM"))
    PT = ctx.enter_context(tc.tile_pool(name="PT", bufs=2, space="PSUM"))

    mm = nc.tensor.matmul

    # ---------------- constants ----------------
    ident = consts.tile([128, 128], f32, name="ident", tag="ident")
    make_identity(nc, ident[:])
    ident_b = consts.tile([128, 128], bf16, name="ident_b", tag="ident_b")
    make_identity(nc, ident_b[:])
    cmpi = consts.tile([128, NT], i32, name="cmpi", tag="cmpi")
    nc.gpsimd.iota(cmpi[:], pattern=[[1, NT]], base=0, channel_multiplier=0)
    cmpf = consts.tile([128, NT], f32, name="cmpf", tag="cmpf")
    nc.vector.tensor_copy(cmpf[:], cmpi[:])
    zeros301b = consts.tile([128, HID], bf16, name="zeros301b", tag="zeros301b")
    nc.vector.memset(zeros301b[:], 0.0)

    # ---------------- weights ----------------
    whh = []
    for k, kk in enumerate([128, 128, 83]):
        t = wp.tile([kk, GI], bf16, name=f"whh{k}", tag=f"whh{k}")
        nc.sync.dma_start(t[:], d[f"whh{k}"][:])
        whh.append(t)
    # inn weights at base partition 64 to match the combo one-hot rows
    wpreI = wp.tile([83, HID], bf16, name="wpreI", tag="wpreI")
    nc.sync.dma_start(wpreI[64:83, :], d["wpreI"][:])
    wgm = []
    for k in range(2):
        t = wp.tile([128, 602], bf16, name=f"wgm{k}", tag=f"wgm{k}")
        nc.sync.dma_start(t[:], d[f"wgm{k}"][:])
        wgm.append(t)
    wBc = wp.tile([109, 602], bf16, name="wBc", tag="wBc")
    nc.sync.dma_start(wBc[:], d["wBc109"][:])
    wfc = []
    for k, sz in enumerate([128, 128, 97]):
        t = wp.tile([sz, OUT_W], bf16, name=f"wfc{k}", tag=f"wfc{k}")
        nc.sync.dma_start(t[:], d["wfc"][128 * k:128 * k + sz, :])
        wfc.append(t)
    wdf1 = wp.tile([33, 16], bf16, name="wdf1", tag="wdf1")
    nc.sync.dma_start(wdf1[:], d["wdf1"][:])
    wdf2 = wp.tile([33, 8], bf16, name="wdf2", tag="wdf2")
    nc.sync.dma_start(wdf2[:], d["wdf2"][:])

    # ---------------- inputs, one-hot ----------------
    adj_f, feats, X19 = [], [], []
    for c in range(CH):
        ti = pin.tile([128, MAX_N], i32, name="ti", tag="ti")
        nc.sync.dma_start(ti[:], d["types"][cs(c), :])
        tf = pin.tile([128, MAX_N], f32, name="tf", tag="tf")
        nc.vector.tensor_copy(tf[:], ti[:])
        pi = pin.tile([128, MAX_N], i32, name="pi", tag="pi")
        nc.sync.dma_start(pi[:], d["paths"][cs(c), :])
        pf = pin.tile([128, MAX_N], f32, name="pf", tag="pf")
        nc.vector.tensor_copy(pf[:], pi[:])
        ai = pin.tile([128, MAX_N * MAX_N], i32, name="ai", tag="ai")
        nc.sync.dma_start(ai[:], d["adj"][cs(c), :])
        af = big.tile([128, MAX_N * MAX_N], f32, name=f"adj{c}", tag=f"adj{c}")
        nc.vector.tensor_copy(af[:], ai[:])
        adj_f.append(af)
        ft = big.tile([128, 3 * MAX_N], f32, name=f"feats{c}", tag=f"feats{c}")
        nc.sync.dma_start(ft[:], d["feats"][cs(c), :])
        feats.append(ft)

        x = big.tile([128, MAX_N * 19], bf16, name=f"X19_{c}", tag=f"X19_{c}")
        x3 = x[:].rearrange("p (v j) -> p v j", j=19)
        nc.vector.tensor_tensor(
            x3[:, :, 0:NT],
            tf[:].unsqueeze(2).broadcast_to([128, MAX_N, NT]),
            cmpf[:, 0:NT].unsqueeze(1).broadcast_to([128, MAX_N, NT]),
            OP.is_equal,
        )
        nc.vector.tensor_tensor(
            x3[:, :, NT:19],
            pf[:].unsqueeze(2).broadcast_to([128, MAX_N, PP]),
            cmpf[:, 0:PP].unsqueeze(1).broadcast_to([128, MAX_N, PP]),
            OP.is_equal,
        )
        X19.append(x)

    # gated message store (written once per vertex, read by later vertices)
    G_all = [big.tile([128, 11 * HID], bf16, name=f"G{c}", tag=f"G{c}")
             for c in range(CH)]
    # x-side candidate pre-activations, filled per step
    inn_all = [big.tile([128, MAX_N * HID], bf16, name=f"inn{c}",
                        tag=f"inn{c}") for c in range(CH)]

    # feature-major h_in / h buffers (k-chunks 0,1; tails live in combo)
    hiT = [big.tile([128, BL], bf16, name="hiT0", tag="hiT0"),
           big.tile([128, BL], bf16, name="hiT1", tag="hiT1")]
    hvT = [big.tile([128, BL], bf16, name="hvT0", tag="hvT0"),
           big.tile([128, BL], bf16, name="hvT1", tag="hvT1")]
    # FC tail lhsT: rows 0:45 h-tail, 64:72 Hd, 96 ones (32-aligned writes)
    fcK2 = big.tile([97, BL], bf16, name="fcK2", tag="fcK2")
    nc.vector.memset(fcK2[:], 0.0)
    nc.vector.memset(fcK2[96:97, :], 1.0)

    # ---------------- the vertex scan ----------------
    for v in range(MAX_N):
        # gh-side combo: rows 0:45 h_in tail, 45:64 zero pad, 64:83 one-hot
        cmbI = p_cmb.tile([83, BL], bf16, name="cmbI", tag="cmbI")
        # gate-side combo: rows 0:19 one-hot, 19:64 zero pad, 64:109 h_v tail
        cmbV = p_cmb.tile([109, BL], bf16, name="cmbV", tag="cmbV")
        if v < 2:  # once per pool buffer: the pad rows stay zero forever
            nc.vector.memset(cmbI[:], 0.0)
            nc.vector.memset(cmbV[:], 0.0)
        # one-hot rows: transpose X19 v-block straight into the combo tiles
        pto = PT.tile([128, BL], bf16, name="ptb", tag="ptb")
        for c in range(CH):
            nc.tensor.transpose(pto[0:19, cs(c)],
                                X19[c][:, 19 * v:19 * v + 19], ident_b[:])
        nc.scalar.copy(cmbI[64:83, :], pto[0:19, :])
        nc.scalar.copy(cmbV[0:19, :], pto[0:19, :])

        # x-side candidate (inn) for this vertex: K=19 one-hot matmul
        for c in range(CH):
            ptI = PB.tile([128, 1024], f32, name="R", tag="R")
            mm(ptI[:, 0:HID], cmbI[64:83, cs(c)], wpreI[64:83, :],
               start=True, stop=True)
            if c % 2 == 0:
                nc.scalar.copy(inn_all[c][:, HID * v:HID * (v + 1)],
                               ptI[:, 0:HID])
            else:
                nc.vector.tensor_copy(inn_all[c][:, HID * v:HID * (v + 1)],
                                      ptI[:, 0:HID])

        # masked sum of predecessor gated messages, then its transpose
        if v > 0:
            h_ins = []
            for c in range(CH):
                hi = p_hin.tile([128, HID], bf16, name="hin", tag="hin")
                a = adj_f[c]
                nc.vector.tensor_scalar_mul(
                    hi[:], G_all[c][:, 0:HID],
                    a[:, MAX_N * v:MAX_N * v + 1])
                for u in range(1, v):
                    nc.vector.scalar_tensor_tensor(
                        hi[:], G_all[c][:, HID * u:HID * (u + 1)],
                        a[:, MAX_N * v + u:MAX_N * v + u + 1], hi[:],
                        op0=OP.mult, op1=OP.add)
                h_ins.append(hi)
            for k, (ko, kk) in enumerate(KCH):
                pt = PT.tile([128, BL], bf16, name="ptb", tag="ptb")
                for c in range(CH):
                    nc.tensor.transpose(pt[0:kk, cs(c)],
                                        h_ins[c][:, ko:ko + kk], ident_b[:])
                if k < 2:
                    nc.scalar.copy(hiT[k][:], pt[0:128, :])
                else:
                    nc.scalar.copy(cmbI[0:45, :], pt[0:45, :])
        else:
            h_ins = [zeros301b] * CH

        # gi+gh fused in PSUM, then the GRU cell elementwise
        hvs = []
        for c in range(CH):
            R = PB.tile([128, 1024], f32, name="R", tag="R")
            if v > 0:
                for k in range(2):
                    l = hiT[k][:, cs(c)]
                    mm(R[:, 0:512], l, whh[k][:, 0:512],
                       start=(k == 0), stop=False)
                    mm(R[:, 512:903], l, whh[k][:, 512:903],
                       start=(k == 0), stop=False)
                l2 = cmbI[0:83, cs(c)]
                mm(R[:, 0:512], l2, whh[2][:, 0:512], start=False, stop=True)
                mm(R[:, 512:903], l2, whh[2][:, 512:903],
                   start=False, stop=True)
            else:
                l2 = cmbI[0:83, cs(c)]
                mm(R[:, 0:512], l2, whh[2][:, 0:512], start=True, stop=True)
                mm(R[:, 512:903], l2, whh[2][:, 512:903],
                   start=True, stop=True)
            rz = p_rz.tile([128, 602], bf16, name="rz", tag="rz")
            nc.scalar.activation(rz[:], R[:, 0:602], AF.Sigmoid)
            tn = p_t.tile([128, HID], bf16, name="tn", tag="tn")
            nc.vector.tensor_mul(tn[:], rz[:, 0:HID], R[:, 602:903])
            tn2 = p_t.tile([128, HID], bf16, name="tn2", tag="tn2")
            nc.vector.tensor_add(tn2[:], tn[:],
                                 inn_all[c][:, HID * v:HID * (v + 1)])
            g = p_g.tile([128, HID], bf16, name="g", tag="g")
            nc.scalar.activation(g[:], tn2[:], AF.Tanh)
            t3 = p_t.tile([128, HID], bf16, name="t3", tag="t3")
            nc.vector.tensor_sub(t3[:], h_ins[c][:], g[:])
            t4 = p_t.tile([128, HID], bf16, name="t4", tag="t4")
            nc.vector.tensor_mul(t4[:], t3[:], rz[:, HID:602])
            hv = p_hv.tile([128, HID], bf16, name="hv", tag="hv")
            nc.vector.tensor_add(hv[:], g[:], t4[:])
            hvs.append(hv)

        # transpose h_v to feature-major (step 11 k=2 lands in the FC tile)
        for k, (ko, kk) in enumerate(KCH):
            pt = PT.tile([128, BL], bf16, name="ptb", tag="ptb")
            for c in range(CH):
                nc.tensor.transpose(pt[0:kk, cs(c)],
                                    hvs[c][:, ko:ko + kk], ident_b[:])
            if k < 2:
                nc.scalar.copy(hvT[k][:], pt[0:128, :])
            elif v == MAX_N - 1:
                nc.scalar.copy(fcK2[0:45, :], pt[0:45, :])
            else:
                nc.scalar.copy(cmbV[64:109, :], pt[0:45, :])

        # gate * mapper on [h_v, pos-onehot]; combo rows 45:109 carry the
        # one-hot (pos+bg via wBc64) and the h_v tail
        if v < MAX_N - 1:
            for c in range(CH):
                P = PB.tile([128, 1024], f32, name="R", tag="R")
                cv = cmbV[0:109, cs(c)]
                mm(P[:, 0:512], cv, wBc[:, 0:512], start=True, stop=False)
                mm(P[:, 512:602], cv, wBc[:, 512:602], start=True, stop=False)
                for k in range(2):
                    l = hvT[k][:, cs(c)]
                    last = k == 1
                    mm(P[:, 0:512], l, wgm[k][:, 0:512],
                       start=False, stop=last)
                    mm(P[:, 512:602], l, wgm[k][:, 512:602],
                       start=False, stop=last)
                sg = p_sg.tile([128, HID], bf16, name="sg", tag="sg")
                nc.scalar.activation(sg[:], P[:, 0:HID], AF.Sigmoid)
                nc.vector.tensor_mul(G_all[c][:, HID * v:HID * (v + 1)],
                                     sg[:], P[:, HID:602])

    # ---------------- df scatter (last write wins) + df MLP ----------------
    dfT = big.tile([33, BL], bf16, name="dfT", tag="dfT")
    nc.vector.memset(dfT[:], 0.0)
    nc.vector.memset(dfT[32:33, :], 1.0)
    for c in range(CH):
        df = big.tile([128, 27], f32, name=f"df{c}", tag=f"df{c}")
        nc.vector.memset(df[:], 0.0)
        df3 = df[:].rearrange("p (q j) -> p q j", j=3)
        for v in range(MAX_N):
            f3 = feats[c][:, 3 * v:3 * v + 3].unsqueeze(1) \
                .broadcast_to([128, PP, 3])
            oh = X19[c][:, 19 * v + NT:19 * v + 19].unsqueeze(2) \
                .broadcast_to([128, PP, 3])
            s = p_t.tile([128, 27], f32, name="dfs", tag="dfs")
            s3 = s[:].rearrange("p (q j) -> p q j", j=3)
            nc.gpsimd.tensor_sub(s3, f3, df3)
            w = p_t.tile([128, 27], f32, name="dfw", tag="dfw")
            w3 = w[:].rearrange("p (q j) -> p q j", j=3)
            nc.gpsimd.tensor_mul(w3, oh, s3)
            nc.gpsimd.tensor_add(df3, df3, w3)
        dfb = p_t.tile([128, 27], bf16, name="dfb", tag="dfb")
        nc.vector.tensor_copy(dfb[:], df[:])
        pt = PT.tile([128, BL], bf16, name="ptb", tag="ptb")
        nc.tensor.transpose(pt[0:27, cs(c)], dfb[:], ident_b[:])
        nc.any.tensor_copy(dfT[0:27, cs(c)], pt[0:27, cs(c)])

    pd1 = PB.tile([128, 1024], f32, name="R", tag="R")
    mm(pd1[0:16, 0:BL], wdf1[:], dfT[:], start=True, stop=True)
    r1T = big.tile([33, BL], bf16, name="r1T", tag="r1T")
    nc.vector.memset(r1T[:], 0.0)
    nc.vector.memset(r1T[32:33, :], 1.0)
    nc.scalar.activation(r1T[0:16, :], pd1[0:16, 0:BL], AF.Relu)
    pd2 = PB.tile([128, 1024], f32, name="R", tag="R")
    mm(pd2[0:8, 0:BL], wdf2[:], r1T[:], start=True, stop=True)
    nc.any.tensor_copy(fcK2[64:72, :], pd2[0:8, 0:BL])

    # ---------------- final fully-connected (mu | logvar) ----------------
    for c in range(CH):
        po = PB.tile([128, 1024], f32, name="R", tag="R")
        mm(po[:, 0:OUT_W], hvT[0][:, cs(c)], wfc[0][:], start=True, stop=False)
        mm(po[:, 0:OUT_W], hvT[1][:, cs(c)], wfc[1][:], start=False, stop=False)
        mm(po[:, 0:OUT_W], fcK2[:, cs(c)], wfc[2][:], start=False, stop=True)
        ob = p_out.tile([128, OUT_W], f32, name="ob", tag="ob")
        nc.any.tensor_copy(ob[:], po[:, 0:OUT_W])
        nc.sync.dma_start(d_out[cs(c), :], ob[:])


def build_nc():
    nc = bacc.Bacc("TRN2", target_bir_lowering=False, debug=False,
                   num_devices=NCORES)
    d = {}
    for name, shape, dt in [
        ("types", [BL, MAX_N], i32),
        ("paths", [BL, MAX_N], i32),
        ("adj", [BL, MAX_N * MAX_N], i32),
        ("feats", [BL, 3 * MAX_N], f32),
        ("whh0", [128, GI], bf16),
        ("whh1", [128, GI], bf16),
        ("whh2", [83, GI], bf16),
        ("wpreI", [19, HID], bf16),
        ("wgm0", [128, 602], bf16),
        ("wgm1", [128, 602], bf16),
        ("wBc109", [109, 602], bf16),
        ("wfc", [353, OUT_W], bf16),
        ("wdf1", [33, 16], bf16),
        ("wdf2", [33, 8], bf16),
    ]:
        d[name] = nc.dram_tensor(name, shape, dt, kind="ExternalInput").ap()
    d_out = nc.dram_tensor("out", [BL, OUT_W], f32, kind="ExternalOutput").ap()
    with tile.TileContext(nc) as tc:
        with ExitStack() as ctx:
            _body(ctx, tc, nc, d, d_out)
    nc.compile()
    return nc


def prepack(inputs):
    ii = {k: np.asarray(v) for k, v in inputs.items()}
    W_ih, b_ih = ii["W_ih"].astype(np.float32), ii["b_ih"].astype(np.float32)
    Wg, bg = ii["Wg"].astype(np.float32), ii["bg"].astype(np.float32)
    Wm = ii["Wm"].astype(np.float32)
    b_hh = ii["b_hh"].astype(np.float32)
    # one-hot-row weights: gi r/z parts (+b_ih+b_hh) in cols [0:602]; cols
    # [602:903] carry only b_hh's candidate part (inn itself is computed
    # separately via wpreI)
    wpreA = W_ih.T.copy()
    wpreA[:, 602:903] = 0.0
    wpreA[:NT, 0:602] += (b_ih + b_hh)[None, 0:602]
    wpreA[:NT, 602:903] += b_hh[None, 602:903]
    wpreI = W_ih.T[:, 602:903].copy()
    wpreI[:NT] += b_ih[None, 602:903]
    wpreB = np.zeros((19, 602), np.float32)
    wpreB[NT:19, 0:HID] = Wg[:, HID:HID + PP].T + bg[None, :]
    wpreB[NT:19, HID:602] = Wm[:, HID:HID + PP].T
    whhT = ii["W_hh"].astype(np.float32).T.copy()       # [301, 903]
    wgm = np.zeros((HID, 602), np.float32)
    wgm[:, 0:HID] = Wg[:, 0:HID].T
    wgm[:, HID:602] = Wm[:, 0:HID].T
    # h-side k2: rows 0:45 = W_hh^T tail, 45:64 zero pad, 64:83 one-hot
    whh2 = np.zeros((83, GI), np.float32)
    whh2[0:45] = whhT[256:301]
    whh2[64:83] = wpreA
    # gates combo: rows 0:19 = one-hot (pos+bg), rows 64:109 = wgm tail
    wBc109 = np.zeros((109, 602), np.float32)
    wBc109[0:19] = wpreB
    wBc109[64:109] = wgm[256:301]
    # FC lhsT rows: [0:256) = h dims 0:256 (two 128-chunks); tail chunk of 97
    # rows: 0:45 h-tail, 64:72 Hd, 96 biases (matches fcK2 on-device layout)
    wfcT1 = ii["W_fc1"].astype(np.float32).T   # [309, 56]
    wfcT2 = ii["W_fc2"].astype(np.float32).T
    wfc = np.zeros((353, OUT_W), np.float32)
    wfc[0:256, 0:56] = wfcT1[0:256]
    wfc[0:256, 56:112] = wfcT2[0:256]
    tail = np.zeros((97, OUT_W), np.float32)
    tail[0:45, 0:56] = wfcT1[256:301]
    tail[0:45, 56:112] = wfcT2[256:301]
    tail[64:72, 0:56] = wfcT1[301:309]
    tail[64:72, 56:112] = wfcT2[301:309]
    tail[96, 0:56] = ii["b_fc1"].astype(np.float32)
    tail[96, 56:112] = ii["b_fc2"].astype(np.float32)
    wfc[256:353] = tail
    wdf1 = np.zeros((33, 16), np.float32)
    wdf1[0:27] = ii["W_df1"].astype(np.float32).T
    wdf1[32] = ii["b_df1"].astype(np.float32)
    wdf2 = np.zeros((33, 8), np.float32)
    wdf2[0:16] = ii["W_df2"].astype(np.float32).T
    wdf2[32] = ii["b_df2"].astype(np.float32)
    import ml_dtypes
    out = dict(whh0=whhT[0:128], whh1=whhT[128:256], whh2=whh2,
               wpreI=wpreI, wgm0=wgm[0:128], wgm1=wgm[128:256],
               wBc109=wBc109, wfc=wfc, wdf1=wdf1, wdf2=wdf2)
    return {k: np.ascontiguousarray(v).astype(ml_dtypes.bfloat16)
            for k, v in out.items()}


def shard_inputs(inputs):
    ii = {k: np.asarray(v) for k, v in inputs.items()}
    w = prepack(ii)
    maps = []
    for i in range(NCORES):
        sl = slice(i * BL, (i + 1) * BL)
        m = dict(
            types=np.ascontiguousarray(ii["types"][sl]).astype(np.int32),
            paths=np.ascontiguousarray(ii["paths"][sl]).astype(np.int32),
            adj=np.ascontiguousarray(
                ii["adj_raw"][sl].reshape(BL, MAX_N * MAX_N)).astype(np.int32),
            feats=np.ascontiguousarray(
                ii["feats"][sl].reshape(BL, 3 * MAX_N)).astype(np.float32),
            **w,
        )
        maps.append(m)
    return maps


def get_nc():
    if "nc" not in _CACHE:
        _CACHE["nc"] = build_nc()
    return _CACHE["nc"]


def kernel(**inputs):
    nc = get_nc()
    maps = shard_inputs(inputs)
    res = run_bass_kernel_spmd(nc, maps, list(range(NCORES)))
    out = np.concatenate([res.results[i]["out"] for i in range(NCORES)], axis=0)
    return np.ascontiguousarray(out.astype(np.float32))
